# revision 10
# baseline (speedup 1.0000x reference)
"""Single-head attention (B=4, N=2048, D=1024), scores scaled by 10.

Sharding: 8 cores = (batch, query-half). Core 2b+h owns queries
[1024h:1024(h+1)] of batch b. K/V projections are computed for the OWN
half only and exchanged with the pair core (2b ^ 1) via an on-chip
AllGather, halving the projection FLOPs vs recomputing the full
sequence per core. Key order is global [h0|h1] (rank order) on every
core, so the SPMD program is identical across cores.

Numerics: everything runs single-pass fp16 (inputs rounded to fp16,
fp32 PSUM accumulation). Measured end-to-end rel err ~6e-3 against the
fp32 reference (gate 2e-2): the x10 score scale amplifies fp16 rounding
of Q/K into the softmax exponent, but the softmax is near one-hot
(score std ~107 after scaling) so only near-tie rows are affected.

Schedule: all weights prefetch to SBUF before the collectives start so
the weight streams never contend with collective DMA traffic; phase
order K proj -> AllGather K -> V proj -> AllGather V -> Q proj puts
both exchanges behind compute. The attention loop is software-
pipelined: while QK(c) runs on the tensor engine, chunk c-1's softmax
runs beside it (running key-max on GpSimd paced with the score
evacuations, rank-1 max/recip broadcasts as tiny matmuls slotted
between QK accumulation groups, exp on the scalar engine), so the PE
alternates QK / sum / PV blocks with near-zero idle.
"""

import numpy as np

B, SEQ, D = 4, 2048, 1024
NQ = 1024          # queries per core (= keys computed per core)
QCH = 256          # attention q-chunk
NCH = NQ // QCH
NCORES = 8
DT = D // 128      # 8 d-tiles
ET = D // 128      # 8 e-tiles
KT = SEQ // 128    # 16 k-tiles
HKT = KT // 2      # 8 own-half k-tiles

_BUILT = {}


def _build():
    if "nc" in _BUILT:
        return _BUILT["nc"]
    from contextlib import ExitStack

    import concourse.bass as bass  # noqa: F401
    import concourse.mybir as mybir
    import concourse.tile as tile
    from concourse import bacc

    dt = mybir.dt
    F32, F16 = dt.float32, dt.float16
    AL = mybir.AluOpType
    EXP = mybir.ActivationFunctionType.Exp
    GROUPS = [[2 * i, 2 * i + 1] for i in range(NCORES // 2)]

    nc = bacc.Bacc("TRN2", target_bir_lowering=False, debug=False)

    # x^T own half, packed [p, t, n]; weights packed so each e-tile DMA is
    # one contiguous 2KB line per partition
    xp_d = nc.dram_tensor("xp", [128, DT * NQ], F16, kind="ExternalInput")
    wq_d = nc.dram_tensor("wq", [128, ET * DT * 128], F16, kind="ExternalInput")
    wk_d = nc.dram_tensor("wk", [128, ET * DT * 128], F16, kind="ExternalInput")
    wv_d = nc.dram_tensor("wv", [128, DT * D], F16, kind="ExternalInput")
    ot_d = nc.dram_tensor("ot", [128, DT * NQ], F16, kind="ExternalOutput")

    xp_r = xp_d.ap().rearrange("p (t n) -> p t n", t=DT)
    wq_r = wq_d.ap().rearrange("p (e tc) -> p e tc", e=ET)
    wk_r = wk_d.ap().rearrange("p (e tc) -> p e tc", e=ET)
    wv_r = wv_d.ap().rearrange("p (t e) -> p t e", t=DT)
    ot_r = ot_d.ap().rearrange("p (t q) -> p t q", t=DT)

    with tile.TileContext(nc) as tc, ExitStack() as ctx:
        qk_pool = ctx.enter_context(tc.tile_pool(name="qk", bufs=1))
        qt = qk_pool.tile([128, ET, NQ], F16, tag="qt")
        ktt = qk_pool.tile([128, ET, SEQ], F16, tag="ktt")
        vf = qk_pool.tile([128, KT, D], F16, tag="vf")

        const_pool = ctx.enter_context(tc.tile_pool(name="const", bufs=1))
        ten32 = const_pool.tile([1, 128], F32, tag="ten32")
        one32 = const_pool.tile([1, 128], F32, tag="one32")
        ones16 = const_pool.tile([128, 1], F16, tag="ones16")
        nc.vector.memset(ten32[:], 10.0)
        nc.vector.memset(one32[:], 1.0)
        nc.vector.memset(ones16[:], 1.0)

        dram = ctx.enter_context(tc.tile_pool(name="dram", bufs=1, space="DRAM"))
        k_in = dram.tile([D, NQ], F16, tag="k_in")
        k_out = dram.tile([2 * D, NQ], F16, tag="k_out")
        v_in = dram.tile([NQ, D], F16, tag="v_in")
        v_out = dram.tile([SEQ, D], F16, tag="v_out")
        warm_in = dram.tile([16, 16], F16, tag="warm_in")
        warm_out = dram.tile([32, 16], F16, tag="warm_out")

        # tiny warmup collective at t=0: pays the ncfw channel-setup latency
        # before the real exchanges need it
        warm_sb = const_pool.tile([16, 16], F16, tag="warm_sb")
        nc.vector.memset(warm_sb[:], 0.0)
        nc.sync.dma_start(warm_in[:], warm_sb[:])
        nc.gpsimd.collective_compute(
            "AllGather",
            AL.bypass,
            replica_groups=GROUPS,
            ins=[warm_in[:]],
            outs=[warm_out[:]],
        )

        # ---------------- Projections (all single-pass fp16) --------------
        with (
            tc.tile_pool(name="xspan", bufs=1) as xspan,
            tc.tile_pool(name="wall", bufs=1) as wall,
            tc.tile_pool(name="kev", bufs=4) as kevpool,
            tc.tile_pool(name="psA", bufs=4, space="PSUM") as psA,
        ):
            x_t = xspan.tile([128, DT, NQ], F16, tag="x")
            wkF = wall.tile([128, ET, DT * 128], F16, tag="wkF")
            wvF = wall.tile([128, DT, D], F16, tag="wvF")
            wqF = wall.tile([128, ET, DT * 128], F16, tag="wqF")
            # DMA order = need order; first tiles split in half so they land
            # on parallel rings and the first matmul starts ASAP
            for hh in range(2):
                nc.sync.dma_start(
                    wkF[:, 0, 512 * hh : 512 * (hh + 1)],
                    wk_r[:, 0, 512 * hh : 512 * (hh + 1)],
                )
            for dti in range(DT):
                for hh in range(2):
                    nc.sync.dma_start(
                        x_t[:, dti, 512 * hh : 512 * (hh + 1)],
                        xp_r[:, dti, 512 * hh : 512 * (hh + 1)],
                    )
            for et in range(1, ET):
                nc.sync.dma_start(wkF[:, et, :], wk_r[:, et, :])
            for ec in range(2):
                nc.sync.dma_start(
                    wvF[:, :, 512 * ec : 512 * (ec + 1)],
                    wv_r[:, :, 512 * ec : 512 * (ec + 1)],
                )
            for et in range(ET):
                nc.sync.dma_start(wqF[:, et, :], wq_r[:, et, :])

            # ---- Phase K: own-half K^T projection -----------------------
            for et in range(ET):
                e0 = 128 * et
                ps0 = psA.tile([128, 512], F32, tag="psA")
                ps1 = psA.tile([128, 512], F32, tag="psA")
                ps = (ps0, ps1)
                # dti outer so the first et paces with the incoming x stream
                for dti in range(DT):
                    for chn in range(2):
                        nc.tensor.matmul(
                            ps[chn][:],
                            wkF[:, et, 128 * dti : 128 * (dti + 1)],
                            x_t[:, dti, 512 * chn : 512 * (chn + 1)],
                            start=(dti == 0),
                            stop=(dti == DT - 1),
                        )
                for chn in range(2):
                    n0 = 512 * chn
                    kev = kevpool.tile([128, 512], F16, tag="kev")
                    nc.vector.tensor_copy(kev[:], ps[chn][:])
                    nc.sync.dma_start(k_in[e0 : e0 + 128, n0 : n0 + 512], kev[:])

            # pair AllGather of K halves; readbacks ride the sync queue so
            # the V AllGather can trigger as soon as K's mesh phase ends
            nc.gpsimd.collective_compute(
                "AllGather",
                AL.bypass,
                replica_groups=GROUPS,
                ins=[k_in[:]],
                outs=[k_out[:]],
            )
            k_out_r = k_out[:].rearrange("(h t p) n -> h p t n", p=128, t=ET)
            for h in range(2):
                for q in range(2):
                    nc.sync.dma_start(
                        ktt[:, 4 * q : 4 * (q + 1), NQ * h : NQ * (h + 1)],
                        k_out_r[h, :, 4 * q : 4 * (q + 1), :],
                    )

            # ---- Phase V: own-half V projection --------------------------
            for ec in range(2):
                e0 = 512 * ec
                for kt in range(HKT):
                    k0 = 128 * kt
                    ps = psA.tile([128, 512], F32, tag="psA")
                    for dti in range(DT):
                        nc.tensor.matmul(
                            ps[:],
                            x_t[:, dti, k0 : k0 + 128],
                            wvF[:, dti, e0 : e0 + 512],
                            start=(dti == 0),
                            stop=(dti == DT - 1),
                        )
                    vev = kevpool.tile([128, 512], F16, tag="kev")
                    nc.vector.tensor_copy(vev[:], ps[:])
                    nc.sync.dma_start(v_in[k0 : k0 + 128, e0 : e0 + 512], vev[:])

            nc.gpsimd.collective_compute(
                "AllGather",
                AL.bypass,
                replica_groups=GROUPS,
                ins=[v_in[:]],
                outs=[v_out[:]],
            )
            v_out_r = v_out[:].rearrange("(h t p) e -> h p t e", p=128, t=HKT)
            for h in range(2):
                for q in range(2):
                    nc.gpsimd.dma_start(
                        vf[:, HKT * h + 4 * q : HKT * h + 4 * (q + 1), :],
                        v_out_r[h, :, 4 * q : 4 * (q + 1), :],
                    )

            # ---- Phase Q: own-half Q^T projection ------------------------
            for et in range(ET):
                for chn in range(2):
                    n0 = 512 * chn
                    ps = psA.tile([128, 512], F32, tag="psA")
                    for dti in range(DT):
                        nc.tensor.matmul(
                            ps[:],
                            wqF[:, et, 128 * dti : 128 * (dti + 1)],
                            x_t[:, dti, n0 : n0 + 512],
                            start=(dti == 0),
                            stop=(dti == DT - 1),
                        )
                    nc.vector.tensor_copy(qt[:, et, n0 : n0 + 512], ps[:])

        # ---------------- Attention, q-chunked, software-pipelined --------
        with (
            tc.tile_pool(name="stp", bufs=2) as stpool,
            tc.tile_pool(name="pp", bufs=2) as ppool,
            tc.tile_pool(name="tree", bufs=2) as treepool,
            tc.tile_pool(name="aux", bufs=2) as auxpool,
            tc.tile_pool(name="osb", bufs=3) as outpool,
            tc.tile_pool(name="psS", bufs=4, space="PSUM") as psS,
            tc.tile_pool(name="psO", bufs=2, space="PSUM") as psO,
            tc.tile_pool(name="psX", bufs=1, space="PSUM") as psX,
            tc.tile_pool(name="psR", bufs=1, space="PSUM") as psR,
        ):
            # per-chunk state threaded across the pipeline
            sts = [None] * NCH   # scores [128, KT, QCH] f32
            pts = [None] * NCH   # exp(10(s-max)) [128, KT, QCH] f16
            m1s = [None] * NCH   # per-query max row, doubled [1, 2*QCH]
            mbs = [None] * NCH   # broadcast 10*max, doubled [128, 2*QCH]

            def tree_fold_max(c):
                # rowwise max over kt as a 4-op flat tree, then partition-
                # reduce via 32-partition folds + DVE 32x32 block transposes
                st = sts[c]
                t8 = treepool.tile([128, 8, QCH], F32, tag="t8", name="t8")
                nc.vector.tensor_max(
                    t8[:].rearrange("p a q -> p (a q)"),
                    st[:, 0:8, :].rearrange("p a q -> p (a q)"),
                    st[:, 8:16, :].rearrange("p a q -> p (a q)"),
                )
                nc.vector.tensor_max(
                    t8[:, 0:4, :].rearrange("p a q -> p (a q)"),
                    t8[:, 0:4, :].rearrange("p a q -> p (a q)"),
                    t8[:, 4:8, :].rearrange("p a q -> p (a q)"),
                )
                nc.vector.tensor_max(
                    t8[:, 0:2, :].rearrange("p a q -> p (a q)"),
                    t8[:, 0:2, :].rearrange("p a q -> p (a q)"),
                    t8[:, 2:4, :].rearrange("p a q -> p (a q)"),
                )
                nc.vector.tensor_max(t8[:, 0, :], t8[:, 0, :], t8[:, 1, :])
                fold4 = treepool.tile([32, 4, QCH], F32, tag="fold4", name="f4")
                for a in range(4):
                    nc.sync.dma_start(
                        fold4[:, a, :], t8[32 * a : 32 * (a + 1), 0, :]
                    )
                nc.vector.tensor_max(fold4[:, 0, :], fold4[:, 0, :], fold4[:, 1, :])
                nc.vector.tensor_max(fold4[:, 2, :], fold4[:, 2, :], fold4[:, 3, :])
                nc.vector.tensor_max(fold4[:, 0, :], fold4[:, 0, :], fold4[:, 2, :])
                t32t = treepool.tile([32, QCH], F32, tag="t32t", name="t32t")
                nc.vector.transpose(t32t[:], fold4[:, 0, :])
                mx32 = treepool.tile([32, 32], F32, tag="mx32", name="mx32")
                nc.vector.memset(mx32[:], 0.0)
                nc.vector.reduce_max(
                    mx32[:, 0 : QCH // 32],
                    t32t[:].rearrange("p (j c) -> p j c", c=32),
                    axis=mybir.AxisListType.X,
                )
                mx32t = treepool.tile([32, 32], F32, tag="mx32t", name="mx32t")
                nc.vector.transpose(mx32t[:], mx32[:])
                # doubled row so pairwise [128, 2*QCH] ops need no broadcast
                m1row = treepool.tile([1, 2 * QCH], F32, tag="m1row", name="m1row")
                nc.sync.dma_start(m1row[0:1, 0:QCH], mx32t[0 : QCH // 32, :])
                nc.sync.dma_start(m1row[0:1, QCH : 2 * QCH], mx32t[0 : QCH // 32, :])
                m1s[c] = m1row

            def head(c, maxb_ps):
                # shift+exp of chunk c in kt-pairs (maxb_ps already doubled)
                maxb = auxpool.tile([128, 2 * QCH], F32, tag="maxb", name="maxb")
                mbs[c] = maxb
                nc.vector.tensor_copy(maxb[:], maxb_ps[:])
                p_t = ppool.tile([128, KT, QCH], F16, tag="p", name="p_t")
                pts[c] = p_t

            def head_pair(c, j):
                st, p_t = sts[c], pts[c]
                sp = st[:, 2 * j : 2 * j + 2, :].rearrange("p a q -> p (a q)")
                nc.vector.scalar_tensor_tensor(
                    sp, sp, 10.0, mbs[c][:], op0=AL.mult, op1=AL.subtract
                )
                nc.scalar.activation(
                    p_t[:, 2 * j : 2 * j + 2, :].rearrange("p a q -> p (a q)"),
                    sp,
                    EXP,
                )

            def qk_block(c, prev):
                # QK of chunk c in kt-pairs sharing a PSUM bank; chunk prev's
                # max-broadcast + shift + exp interleave into the streams
                q0 = QCH * c
                st = stpool.tile([128, KT, QCH], F32, tag="st", name="st")
                sts[c] = st
                for j in range(KT // 2):
                    ps = psS.tile([128, 2 * QCH], F32, tag="psS", name="ps")
                    for half in range(2):
                        kt = 2 * j + half
                        k0 = 128 * kt
                        for et in range(ET):
                            nc.tensor.matmul(
                                ps[:, QCH * half : QCH * (half + 1)],
                                ktt[:, et, k0 : k0 + 128],
                                qt[:, et, q0 : q0 + QCH],
                                start=(et == 0),
                                stop=(et == ET - 1),
                            )
                    if prev is not None and j == 1:
                        maxb_ps = psX.tile(
                            [128, 2 * QCH], F32, tag="bc", name="mb"
                        )
                        nc.tensor.matmul(
                            maxb_ps[:], ten32[:], m1s[prev][:],
                            start=True, stop=True,
                        )
                    nc.vector.tensor_copy(
                        st[:, 2 * j : 2 * j + 2, :].rearrange("p a q -> p (a q)"),
                        ps[:],
                    )
                    if prev is not None:
                        if j == 1:
                            head(prev, maxb_ps)
                        if j >= 2:
                            head_pair(prev, j - 2)
                            if j == KT // 2 - 1:
                                head_pair(prev, j - 1)
                                head_pair(prev, j)

            def sums_pv(c):
                # key-sums of exp as rank-1 ones matmuls, then PV in
                # dti-pairs sharing a PSUM bank
                q0 = QCH * c
                p_t = pts[c]
                sum_ps = psR.tile([1, QCH], F32, tag="sum", name="sum_ps")
                for kt in range(KT):
                    nc.tensor.matmul(
                        sum_ps[:],
                        ones16[:],
                        p_t[:, kt, :],
                        start=(kt == 0),
                        stop=(kt == KT - 1),
                    )
                recrow = treepool.tile([1, 2 * QCH], F32, tag="recrow", name="rr")
                nc.vector.reciprocal(recrow[0:1, 0:QCH], sum_ps[:])
                nc.vector.reciprocal(recrow[0:1, QCH : 2 * QCH], sum_ps[:])
                recb_ps = psX.tile([128, 2 * QCH], F32, tag="bc", name="rb")
                nc.tensor.matmul(
                    recb_ps[:], one32[:], recrow[:], start=True, stop=True
                )
                recb = auxpool.tile([128, 2 * QCH], F32, tag="recb", name="recb")
                nc.vector.tensor_copy(recb[:], recb_ps[:])
                for dj in range(DT // 2):
                    ops = psO.tile([128, 2 * QCH], F32, tag="psO", name="ops")
                    for half in range(2):
                        d0 = 128 * (2 * dj + half)
                        for kt in range(KT):
                            nc.tensor.matmul(
                                ops[:, QCH * half : QCH * (half + 1)],
                                vf[:, kt, d0 : d0 + 128],
                                p_t[:, kt, :],
                                start=(kt == 0),
                                stop=(kt == KT - 1),
                            )
                    osb = outpool.tile([128, 2 * QCH], F16, tag="osb", name="osb")
                    nc.vector.scalar_tensor_tensor(
                        osb[:], ops[:], 1.0, recb[:], op0=AL.mult, op1=AL.mult
                    )
                    nc.sync.dma_start(
                        ot_r[:, 2 * dj : 2 * dj + 2, q0 : q0 + QCH],
                        osb[:].rearrange("p (a q) -> p a q", a=2),
                    )

            # software pipeline:
            #   qk(0); tree_fold(0)
            #   qk(1)+head(0); sums_pv(0); tree_fold(1)
            #   qk(2)+head(1); sums_pv(1); tree_fold(2)
            #   qk(3)+head(2); head(3) hoisted; sums_pv(2); sums_pv(3)
            qk_block(0, None)
            tree_fold_max(0)
            for c in range(1, NCH):
                qk_block(c, c - 1)
                if c == NCH - 1:
                    # last chunk's softmax hoisted before PV(c-1) so its exp
                    # is ready when the PE drains
                    tree_fold_max(c)
                    maxb_ps = psX.tile([128, 2 * QCH], F32, tag="bc", name="mb2")
                    nc.tensor.matmul(
                        maxb_ps[:], ten32[:], m1s[c][:], start=True, stop=True
                    )
                    head(c, maxb_ps)
                    for j in range(KT // 2):
                        head_pair(c, j)
                sums_pv(c - 1)
                if c < NCH - 1:
                    tree_fold_max(c)
            sums_pv(NCH - 1)

    nc.compile()
    _BUILT["nc"] = nc
    return nc


def _prep_inputs(x, q_w, k_w, v_w):
    f16 = np.float16

    def pack_w_lhsT(w):
        # w is [out=e, in=d]; pack [p, eb, t, c] = w[eb*128+c, t*128+p]
        a = w.T.astype(f16).reshape(DT, 128, ET, 128)
        return np.ascontiguousarray(a.transpose(1, 2, 0, 3)).reshape(
            128, ET * DT * 128
        )

    def pack_w_rhs(w):
        # pack [p, t, e] = w.T[t*128+p, e]
        a = w.T.astype(f16).reshape(DT, 128, D)
        return np.ascontiguousarray(a.transpose(1, 0, 2)).reshape(128, DT * D)

    wq = pack_w_lhsT(q_w)
    wk = pack_w_lhsT(k_w)
    wv = pack_w_rhs(v_w)

    in_maps = []
    for core in range(NCORES):
        b, h = divmod(core, 2)
        xt = np.asarray(x[b, NQ * h : NQ * (h + 1)]).T.astype(f16)  # [d, n]
        xp = np.ascontiguousarray(
            xt.reshape(DT, 128, NQ).transpose(1, 0, 2)
        ).reshape(128, DT * NQ)
        in_maps.append({"xp": xp, "wq": wq, "wk": wk, "wv": wv})
    return in_maps


def run(x, q_w, k_w, v_w, trace=False):
    from concourse.bass_utils import run_bass_kernel_spmd

    nc = _build()
    in_maps = _prep_inputs(x, q_w, k_w, v_w)
    res = run_bass_kernel_spmd(nc, in_maps, list(range(NCORES)), trace=trace)
    out = np.empty((B, SEQ, D), np.float32)
    for core in range(NCORES):
        b, h = divmod(core, 2)
        ot = res.results[core]["ot"].astype(np.float32).reshape(128, DT, NQ)
        out[b, NQ * h : NQ * (h + 1)] = (
            ot.transpose(1, 0, 2).reshape(D, NQ).T
        )
    return out, res


def kernel(x, q_w, k_w, v_w):
    x = np.asarray(x, np.float32)
    q_w = np.asarray(q_w, np.float32)
    k_w = np.asarray(k_w, np.float32)
    v_w = np.asarray(v_w, np.float32)
    out, _ = run(x, q_w, k_w, v_w, trace=False)
    return out


if __name__ == "__main__":
    rng = np.random.default_rng(0)
    x = rng.standard_normal((B, SEQ, D), np.float32)
    s = 1.0 / np.sqrt(D)
    q_w = rng.uniform(-s, s, (D, D)).astype(np.float32)
    k_w = rng.uniform(-s, s, (D, D)).astype(np.float32)
    v_w = rng.uniform(-s, s, (D, D)).astype(np.float32)
    out = kernel(x, q_w, k_w, v_w)
    print(out.shape, out.dtype)


# revision 12
# speedup vs baseline: 1.1133x; 1.1133x over previous
"""Single-head attention (B=4, N=2048, D=1024), scores scaled by 10.

Sharding: 8 cores = (batch, query-half). Core 2b+h owns queries
[1024h:1024(h+1)] of batch b. K/V projections are computed for the OWN
half only and exchanged with the pair core (2b ^ 1) via an on-chip
AllGather, halving the projection FLOPs vs recomputing the full
sequence per core. Key order is global [h0|h1] (rank order) on every
core, so the SPMD program is identical across cores.

Numerics: everything runs single-pass fp16 (inputs rounded to fp16,
fp32 PSUM accumulation). Measured end-to-end rel err ~6e-3 against the
fp32 reference (gate 2e-2): the x10 score scale amplifies fp16 rounding
of Q/K into the softmax exponent, but the softmax is near one-hot
(score std ~107 after scaling) so only near-tie rows are affected.

Schedule: all weights prefetch to SBUF before the collectives start so
the weight streams never contend with collective DMA traffic; phase
order K proj -> AllGather K -> V proj -> AllGather V -> Q proj puts
both exchanges behind compute. The attention loop is software-
pipelined: while QK(c) runs on the tensor engine, chunk c-1's softmax
runs beside it (running key-max on GpSimd paced with the score
evacuations, rank-1 max/recip broadcasts as tiny matmuls slotted
between QK accumulation groups, exp on the scalar engine), so the PE
alternates QK / sum / PV blocks with near-zero idle.
"""

import numpy as np

B, SEQ, D = 4, 2048, 1024
NQ = 1024          # queries per core (= keys computed per core)
QCH = 256          # attention q-chunk
NCH = NQ // QCH
NCORES = 8
DT = D // 128      # 8 d-tiles
ET = D // 128      # 8 e-tiles
KT = SEQ // 128    # 16 k-tiles
HKT = KT // 2      # 8 own-half k-tiles

_BUILT = {}


def _build():
    if "nc" in _BUILT:
        return _BUILT["nc"]
    from contextlib import ExitStack

    import concourse.bass as bass  # noqa: F401
    import concourse.mybir as mybir
    import concourse.tile as tile
    from concourse import bacc

    dt = mybir.dt
    F32, F16 = dt.float32, dt.float16
    AL = mybir.AluOpType
    EXP = mybir.ActivationFunctionType.Exp
    GROUPS = [[2 * i, 2 * i + 1] for i in range(NCORES // 2)]

    nc = bacc.Bacc("TRN2", target_bir_lowering=False, debug=False)

    # x^T own half, packed [p, t, n]; weights packed so each e-tile DMA is
    # one contiguous 2KB line per partition
    xp_d = nc.dram_tensor("xp", [128, DT * NQ], F16, kind="ExternalInput")
    wq_d = nc.dram_tensor("wq", [128, ET * DT * 128], F16, kind="ExternalInput")
    wk_d = nc.dram_tensor("wk", [128, ET * DT * 128], F16, kind="ExternalInput")
    wv_d = nc.dram_tensor("wv", [128, DT * D], F16, kind="ExternalInput")
    ot_d = nc.dram_tensor("ot", [128, DT * NQ], F16, kind="ExternalOutput")

    xp_r = xp_d.ap().rearrange("p (t n) -> p t n", t=DT)
    wq_r = wq_d.ap().rearrange("p (e tc) -> p e tc", e=ET)
    wk_r = wk_d.ap().rearrange("p (e tc) -> p e tc", e=ET)
    wv_r = wv_d.ap().rearrange("p (t e) -> p t e", t=DT)
    ot_r = ot_d.ap().rearrange("p (t q) -> p t q", t=DT)

    with tile.TileContext(nc) as tc, ExitStack() as ctx:
        qk_pool = ctx.enter_context(tc.tile_pool(name="qk", bufs=1))
        qt = qk_pool.tile([128, ET, NQ], F16, tag="qt")
        ktt = qk_pool.tile([128, ET, SEQ], F16, tag="ktt")
        vf = qk_pool.tile([128, KT, D], F16, tag="vf")

        const_pool = ctx.enter_context(tc.tile_pool(name="const", bufs=1))
        ten32 = const_pool.tile([1, 128], F32, tag="ten32")
        one32 = const_pool.tile([1, 128], F32, tag="one32")
        ones16 = const_pool.tile([128, 1], F16, tag="ones16")
        nc.vector.memset(ten32[:], 10.0)
        nc.vector.memset(one32[:], 1.0)
        nc.vector.memset(ones16[:], 1.0)

        dram = ctx.enter_context(tc.tile_pool(name="dram", bufs=1, space="DRAM"))
        k_in = dram.tile([D, NQ], F16, tag="k_in")
        k_out = dram.tile([2 * D, NQ], F16, tag="k_out")
        v_in = dram.tile([NQ, D], F16, tag="v_in")
        v_out = dram.tile([SEQ, D], F16, tag="v_out")
        warm_in = dram.tile([16, 16], F16, tag="warm_in")
        warm_out = dram.tile([32, 16], F16, tag="warm_out")

        # tiny warmup collective at t=0: pays the ncfw channel-setup latency
        # before the real exchanges need it
        warm_sb = const_pool.tile([16, 16], F16, tag="warm_sb")
        nc.vector.memset(warm_sb[:], 0.0)
        nc.sync.dma_start(warm_in[:], warm_sb[:])
        nc.gpsimd.collective_compute(
            "AllGather",
            AL.bypass,
            replica_groups=GROUPS,
            ins=[warm_in[:]],
            outs=[warm_out[:]],
        )

        # ---------------- Projections (all single-pass fp16) --------------
        with (
            tc.tile_pool(name="xspan", bufs=1) as xspan,
            tc.tile_pool(name="wall", bufs=1) as wall,
            tc.tile_pool(name="kev", bufs=4) as kevpool,
            tc.tile_pool(name="psA", bufs=4, space="PSUM") as psA,
        ):
            x_t = xspan.tile([128, DT, NQ], F16, tag="x")
            wkF = wall.tile([128, ET, DT * 128], F16, tag="wkF")
            wvF = wall.tile([128, DT, D], F16, tag="wvF")
            wqF = wall.tile([128, ET, DT * 128], F16, tag="wqF")
            # dma_start dispatch serializes at ~1us per call on the issuing
            # engine, and a ring delivers ~one 2KB line per 110ns, so the
            # loads are split by partition-half for ring parallelism and
            # spread across the sync/vector/scalar queues (all idle here)
            nc.scalar.dma_start(wkF[0:64, 0, :], wk_r[0:64, 0, :])
            nc.scalar.dma_start(wkF[64:128, 0, :], wk_r[64:128, 0, :])
            for dti in range(DT):
                nc.sync.dma_start(x_t[0:64, dti, :], xp_r[0:64, dti, :])
                nc.scalar.dma_start(x_t[64:128, dti, :], xp_r[64:128, dti, :])
            for et in range(1, ET):
                nc.scalar.dma_start(wkF[0:64, et, :], wk_r[0:64, et, :])
                nc.scalar.dma_start(wkF[64:128, et, :], wk_r[64:128, et, :])
            for ec in range(2):
                nc.scalar.dma_start(
                    wvF[:, :, 512 * ec : 512 * (ec + 1)],
                    wv_r[:, :, 512 * ec : 512 * (ec + 1)],
                )
            for et in range(ET):
                nc.scalar.dma_start(wqF[:, et, :], wq_r[:, et, :])

            # ---- Phase K: own-half K^T projection -----------------------
            for et in range(ET):
                e0 = 128 * et
                ps0 = psA.tile([128, 512], F32, tag="psA")
                ps1 = psA.tile([128, 512], F32, tag="psA")
                ps = (ps0, ps1)
                # dti outer so the first et paces with the incoming x stream
                for dti in range(DT):
                    for chn in range(2):
                        nc.tensor.matmul(
                            ps[chn][:],
                            wkF[:, et, 128 * dti : 128 * (dti + 1)],
                            x_t[:, dti, 512 * chn : 512 * (chn + 1)],
                            start=(dti == 0),
                            stop=(dti == DT - 1),
                        )
                kev = kevpool.tile([128, NQ], F16, tag="kev")
                for chn in range(2):
                    nc.vector.tensor_copy(
                        kev[:, 512 * chn : 512 * (chn + 1)], ps[chn][:]
                    )
                for ph in range(2):
                    p0 = 64 * ph
                    nc.sync.dma_start(
                        k_in[e0 + p0 : e0 + p0 + 64, :], kev[p0 : p0 + 64, :]
                    )

            nc.gpsimd.collective_compute(
                "AllGather",
                AL.bypass,
                replica_groups=GROUPS,
                ins=[k_in[:]],
                outs=[k_out[:]],
            )

            # ---- Phase V: own-half V projection --------------------------
            # kt outer (weights are resident) so each key row writes back as
            # one full-width 2KB-line DMA
            for kt in range(HKT):
                k0 = 128 * kt
                ps0 = psA.tile([128, 512], F32, tag="psA")
                ps1 = psA.tile([128, 512], F32, tag="psA")
                ps = (ps0, ps1)
                for dti in range(DT):
                    for ec in range(2):
                        nc.tensor.matmul(
                            ps[ec][:],
                            x_t[:, dti, k0 : k0 + 128],
                            wvF[:, dti, 512 * ec : 512 * (ec + 1)],
                            start=(dti == 0),
                            stop=(dti == DT - 1),
                        )
                vev = kevpool.tile([128, D], F16, tag="kev")
                for ec in range(2):
                    nc.vector.tensor_copy(
                        vev[:, 512 * ec : 512 * (ec + 1)], ps[ec][:]
                    )
                for ph in range(2):
                    p0 = 64 * ph
                    nc.sync.dma_start(
                        v_in[k0 + p0 : k0 + p0 + 64, :], vev[p0 : p0 + 64, :]
                    )

            nc.gpsimd.collective_compute(
                "AllGather",
                AL.bypass,
                replica_groups=GROUPS,
                ins=[v_in[:]],
                outs=[v_out[:]],
            )

            # collective readbacks: issued after all v_in writes so their
            # ring descriptors (blocked on the AllGather semaphores) never
            # sit ahead of traffic the collectives themselves need.
            # 16 DMAs each across the rings: ~4MB in ~14us once the
            # exchange lands. ktt on sync (free now), vf on scalar.
            k_out_r = k_out[:].rearrange("(h t p) n -> h p t n", p=128, t=ET)
            for h in range(2):
                for q in range(4):
                    t0 = 2 * q
                    for ph in range(2):
                        p0 = 64 * ph
                        nc.sync.dma_start(
                            ktt[p0 : p0 + 64, t0 : t0 + 2, NQ * h : NQ * (h + 1)],
                            k_out_r[h, p0 : p0 + 64, t0 : t0 + 2, :],
                        )
            v_out_r = v_out[:].rearrange("(h t p) e -> h p t e", p=128, t=HKT)
            for h in range(2):
                for q in range(4):
                    t0 = 2 * q
                    for ph in range(2):
                        p0 = 64 * ph
                        nc.scalar.dma_start(
                            vf[p0 : p0 + 64, HKT * h + t0 : HKT * h + t0 + 2, :],
                            v_out_r[h, p0 : p0 + 64, t0 : t0 + 2, :],
                        )

            # ---- Phase Q: own-half Q^T projection ------------------------
            for et in range(ET):
                for chn in range(2):
                    n0 = 512 * chn
                    ps = psA.tile([128, 512], F32, tag="psA")
                    for dti in range(DT):
                        nc.tensor.matmul(
                            ps[:],
                            wqF[:, et, 128 * dti : 128 * (dti + 1)],
                            x_t[:, dti, n0 : n0 + 512],
                            start=(dti == 0),
                            stop=(dti == DT - 1),
                        )
                    nc.vector.tensor_copy(qt[:, et, n0 : n0 + 512], ps[:])

        # ---------------- Attention, q-chunked, software-pipelined --------
        with (
            tc.tile_pool(name="stp", bufs=2) as stpool,
            tc.tile_pool(name="pp", bufs=2) as ppool,
            tc.tile_pool(name="tree", bufs=2) as treepool,
            tc.tile_pool(name="aux", bufs=2) as auxpool,
            tc.tile_pool(name="osb", bufs=3) as outpool,
            tc.tile_pool(name="psS", bufs=4, space="PSUM") as psS,
            tc.tile_pool(name="psO", bufs=2, space="PSUM") as psO,
            tc.tile_pool(name="psX", bufs=1, space="PSUM") as psX,
            tc.tile_pool(name="psR", bufs=1, space="PSUM") as psR,
        ):
            # per-chunk state threaded across the pipeline
            sts = [None] * NCH   # scores [128, KT, QCH] f32
            pts = [None] * NCH   # exp(10(s-max)) [128, KT, QCH] f16
            m1s = [None] * NCH   # per-query max row, doubled [1, 2*QCH]
            mbs = [None] * NCH   # broadcast 10*max, doubled [128, 2*QCH]

            def tree_fold_max(c):
                # rowwise max over kt as a 4-op flat tree, then partition-
                # reduce via 32-partition folds + DVE 32x32 block transposes
                st = sts[c]
                t8 = treepool.tile([128, 8, QCH], F32, tag="t8", name="t8")
                nc.vector.tensor_max(
                    t8[:].rearrange("p a q -> p (a q)"),
                    st[:, 0:8, :].rearrange("p a q -> p (a q)"),
                    st[:, 8:16, :].rearrange("p a q -> p (a q)"),
                )
                nc.vector.tensor_max(
                    t8[:, 0:4, :].rearrange("p a q -> p (a q)"),
                    t8[:, 0:4, :].rearrange("p a q -> p (a q)"),
                    t8[:, 4:8, :].rearrange("p a q -> p (a q)"),
                )
                nc.vector.tensor_max(
                    t8[:, 0:2, :].rearrange("p a q -> p (a q)"),
                    t8[:, 0:2, :].rearrange("p a q -> p (a q)"),
                    t8[:, 2:4, :].rearrange("p a q -> p (a q)"),
                )
                nc.vector.tensor_max(t8[:, 0, :], t8[:, 0, :], t8[:, 1, :])
                fold4 = treepool.tile([32, 4, QCH], F32, tag="fold4", name="f4")
                for a in range(4):
                    nc.sync.dma_start(
                        fold4[:, a, :], t8[32 * a : 32 * (a + 1), 0, :]
                    )
                nc.vector.tensor_max(fold4[:, 0, :], fold4[:, 0, :], fold4[:, 1, :])
                nc.vector.tensor_max(fold4[:, 2, :], fold4[:, 2, :], fold4[:, 3, :])
                nc.vector.tensor_max(fold4[:, 0, :], fold4[:, 0, :], fold4[:, 2, :])
                t32t = treepool.tile([32, QCH], F32, tag="t32t", name="t32t")
                nc.vector.transpose(t32t[:], fold4[:, 0, :])
                mx32 = treepool.tile([32, 32], F32, tag="mx32", name="mx32")
                nc.vector.memset(mx32[:], 0.0)
                nc.vector.reduce_max(
                    mx32[:, 0 : QCH // 32],
                    t32t[:].rearrange("p (j c) -> p j c", c=32),
                    axis=mybir.AxisListType.X,
                )
                mx32t = treepool.tile([32, 32], F32, tag="mx32t", name="mx32t")
                nc.vector.transpose(mx32t[:], mx32[:])
                # doubled row so pairwise [128, 2*QCH] ops need no broadcast
                m1row = treepool.tile([1, 2 * QCH], F32, tag="m1row", name="m1row")
                nc.sync.dma_start(m1row[0:1, 0:QCH], mx32t[0 : QCH // 32, :])
                nc.sync.dma_start(m1row[0:1, QCH : 2 * QCH], mx32t[0 : QCH // 32, :])
                m1s[c] = m1row

            def head(c, maxb_ps):
                # shift+exp of chunk c in kt-pairs (maxb_ps already doubled)
                maxb = auxpool.tile([128, 2 * QCH], F32, tag="maxb", name="maxb")
                mbs[c] = maxb
                nc.vector.tensor_copy(maxb[:], maxb_ps[:])
                p_t = ppool.tile([128, KT, QCH], F16, tag="p", name="p_t")
                pts[c] = p_t

            def head_pair(c, j):
                st, p_t = sts[c], pts[c]
                sp = st[:, 2 * j : 2 * j + 2, :].rearrange("p a q -> p (a q)")
                nc.vector.scalar_tensor_tensor(
                    sp, sp, 10.0, mbs[c][:], op0=AL.mult, op1=AL.subtract
                )
                nc.scalar.activation(
                    p_t[:, 2 * j : 2 * j + 2, :].rearrange("p a q -> p (a q)"),
                    sp,
                    EXP,
                )

            def qk_block(c, prev):
                # QK of chunk c in kt-pairs sharing a PSUM bank; chunk prev's
                # max-broadcast + shift + exp interleave into the streams
                q0 = QCH * c
                st = stpool.tile([128, KT, QCH], F32, tag="st", name="st")
                sts[c] = st
                for j in range(KT // 2):
                    ps = psS.tile([128, 2 * QCH], F32, tag="psS", name="ps")
                    for half in range(2):
                        kt = 2 * j + half
                        k0 = 128 * kt
                        for et in range(ET):
                            nc.tensor.matmul(
                                ps[:, QCH * half : QCH * (half + 1)],
                                ktt[:, et, k0 : k0 + 128],
                                qt[:, et, q0 : q0 + QCH],
                                start=(et == 0),
                                stop=(et == ET - 1),
                            )
                    if prev is not None and j == 1:
                        maxb_ps = psX.tile(
                            [128, 2 * QCH], F32, tag="bc", name="mb"
                        )
                        nc.tensor.matmul(
                            maxb_ps[:], ten32[:], m1s[prev][:],
                            start=True, stop=True,
                        )
                    nc.vector.tensor_copy(
                        st[:, 2 * j : 2 * j + 2, :].rearrange("p a q -> p (a q)"),
                        ps[:],
                    )
                    if prev is not None:
                        if j == 1:
                            head(prev, maxb_ps)
                        if j >= 2:
                            head_pair(prev, j - 2)
                            if j == KT // 2 - 1:
                                head_pair(prev, j - 1)
                                head_pair(prev, j)

            def sums_pv(c):
                # key-sums of exp as rank-1 ones matmuls, then PV in
                # dti-pairs sharing a PSUM bank
                q0 = QCH * c
                p_t = pts[c]
                sum_ps = psR.tile([1, QCH], F32, tag="sum", name="sum_ps")
                for kt in range(KT):
                    nc.tensor.matmul(
                        sum_ps[:],
                        ones16[:],
                        p_t[:, kt, :],
                        start=(kt == 0),
                        stop=(kt == KT - 1),
                    )
                recrow = treepool.tile([1, 2 * QCH], F32, tag="recrow", name="rr")
                nc.vector.reciprocal(recrow[0:1, 0:QCH], sum_ps[:])
                nc.vector.reciprocal(recrow[0:1, QCH : 2 * QCH], sum_ps[:])
                recb_ps = psX.tile([128, 2 * QCH], F32, tag="bc", name="rb")
                nc.tensor.matmul(
                    recb_ps[:], one32[:], recrow[:], start=True, stop=True
                )
                recb = auxpool.tile([128, 2 * QCH], F32, tag="recb", name="recb")
                nc.vector.tensor_copy(recb[:], recb_ps[:])
                for dj in range(DT // 2):
                    ops = psO.tile([128, 2 * QCH], F32, tag="psO", name="ops")
                    for half in range(2):
                        d0 = 128 * (2 * dj + half)
                        for kt in range(KT):
                            nc.tensor.matmul(
                                ops[:, QCH * half : QCH * (half + 1)],
                                vf[:, kt, d0 : d0 + 128],
                                p_t[:, kt, :],
                                start=(kt == 0),
                                stop=(kt == KT - 1),
                            )
                    osb = outpool.tile([128, 2 * QCH], F16, tag="osb", name="osb")
                    nc.vector.scalar_tensor_tensor(
                        osb[:], ops[:], 1.0, recb[:], op0=AL.mult, op1=AL.mult
                    )
                    nc.sync.dma_start(
                        ot_r[:, 2 * dj : 2 * dj + 2, q0 : q0 + QCH],
                        osb[:].rearrange("p (a q) -> p a q", a=2),
                    )

            # software pipeline:
            #   qk(0); tree_fold(0)
            #   qk(1)+head(0); sums_pv(0); tree_fold(1)
            #   qk(2)+head(1); sums_pv(1); tree_fold(2)
            #   qk(3)+head(2); head(3) hoisted; sums_pv(2); sums_pv(3)
            qk_block(0, None)
            tree_fold_max(0)
            for c in range(1, NCH):
                qk_block(c, c - 1)
                if c == NCH - 1:
                    # last chunk's softmax hoisted before PV(c-1) so its exp
                    # is ready when the PE drains
                    tree_fold_max(c)
                    maxb_ps = psX.tile([128, 2 * QCH], F32, tag="bc", name="mb2")
                    nc.tensor.matmul(
                        maxb_ps[:], ten32[:], m1s[c][:], start=True, stop=True
                    )
                    head(c, maxb_ps)
                    for j in range(KT // 2):
                        head_pair(c, j)
                sums_pv(c - 1)
                if c < NCH - 1:
                    tree_fold_max(c)
            sums_pv(NCH - 1)

    nc.compile()
    _BUILT["nc"] = nc
    return nc


def _prep_inputs(x, q_w, k_w, v_w):
    f16 = np.float16

    def pack_w_lhsT(w):
        # w is [out=e, in=d]; pack [p, eb, t, c] = w[eb*128+c, t*128+p]
        a = w.T.astype(f16).reshape(DT, 128, ET, 128)
        return np.ascontiguousarray(a.transpose(1, 2, 0, 3)).reshape(
            128, ET * DT * 128
        )

    def pack_w_rhs(w):
        # pack [p, t, e] = w.T[t*128+p, e]
        a = w.T.astype(f16).reshape(DT, 128, D)
        return np.ascontiguousarray(a.transpose(1, 0, 2)).reshape(128, DT * D)

    wq = pack_w_lhsT(q_w)
    wk = pack_w_lhsT(k_w)
    wv = pack_w_rhs(v_w)

    in_maps = []
    for core in range(NCORES):
        b, h = divmod(core, 2)
        xt = np.asarray(x[b, NQ * h : NQ * (h + 1)]).T.astype(f16)  # [d, n]
        xp = np.ascontiguousarray(
            xt.reshape(DT, 128, NQ).transpose(1, 0, 2)
        ).reshape(128, DT * NQ)
        in_maps.append({"xp": xp, "wq": wq, "wk": wk, "wv": wv})
    return in_maps


def run(x, q_w, k_w, v_w, trace=False):
    from concourse.bass_utils import run_bass_kernel_spmd

    nc = _build()
    in_maps = _prep_inputs(x, q_w, k_w, v_w)
    res = run_bass_kernel_spmd(nc, in_maps, list(range(NCORES)), trace=trace)
    out = np.empty((B, SEQ, D), np.float32)
    for core in range(NCORES):
        b, h = divmod(core, 2)
        ot = res.results[core]["ot"].astype(np.float32).reshape(128, DT, NQ)
        out[b, NQ * h : NQ * (h + 1)] = (
            ot.transpose(1, 0, 2).reshape(D, NQ).T
        )
    return out, res


def kernel(x, q_w, k_w, v_w):
    x = np.asarray(x, np.float32)
    q_w = np.asarray(q_w, np.float32)
    k_w = np.asarray(k_w, np.float32)
    v_w = np.asarray(v_w, np.float32)
    out, _ = run(x, q_w, k_w, v_w, trace=False)
    return out


if __name__ == "__main__":
    rng = np.random.default_rng(0)
    x = rng.standard_normal((B, SEQ, D), np.float32)
    s = 1.0 / np.sqrt(D)
    q_w = rng.uniform(-s, s, (D, D)).astype(np.float32)
    k_w = rng.uniform(-s, s, (D, D)).astype(np.float32)
    v_w = rng.uniform(-s, s, (D, D)).astype(np.float32)
    out = kernel(x, q_w, k_w, v_w)
    print(out.shape, out.dtype)
